# revision 20
# baseline (speedup 1.0000x reference)
"""Multi-head attention (B=2, T=2048, D=OUT=1024, H=16) on 8 TRN2 NeuronCores.

Sharding: data-parallel over batch (2 groups) x tensor-parallel over heads
(4 groups of 4 heads). Core c handles batch c//4, heads (c%4)*4..(c%4)*4+4.
Each core computes Q^T/K^T/V for its head group, streams softmax(QK^T)V
in transposed layout (keys on partitions), and a partial output projection
through its W_o row block. The host sums the 4 partials per batch and adds
b_o.

Device-side layout notes:
- x is fed transposed ([D, T]) so Q^T/K^T come straight out of the PE.
- The K projection psum is split-written directly into the per-head
  zero-padded kth tiles (no staging copy): head 2mi keeps psum rows 0:64,
  head 2mi+1 keeps rows 64:128, matching where the paired Q^T tile holds
  that head's rows. Every attention matmul then contracts K=128 (K=64
  matmuls do not register as PE activity for the HAM clock gate and run
  at half clock forever).
- The PV stationary is a full 128-column tile per (kt, head) so LDWEIGHTS
  gets fast-weight-load; the softmax denominator rides along as a ones
  column inside it. Per-head column placement is parity-asymmetric so the
  PV output lands directly at the at2p partitions that head occupies:
    even h: V at cols 0:64 (out rows 0:64), ones at col 64 (den at p64)
    odd  h: ones at col 0 (den at p0), V at cols 64:128 (out rows 64:128)
  Unused columns are zeroed once at startup. The odd-head path needs no
  cross-partition DMA hop at all: reciprocal reads the den row straight
  from psum partition 0 and partition_broadcast fills p64:128.
- Matmul operands are bf16 (fp32 PSUM accumulation): full PE clock and
  fast weight load; fp32r measured 2x slower.
- Output partials are written as fp16 ([OUT, T]) -- halves the outbound
  DMA; the host accumulates in fp32 (adds ~1e-4 relative error).
- Emission is one globally software-pipelined stream over all 128
  attention units (PV of unit u-1 emitted inside unit u, across head
  boundaries; attn psum bufs=2 keeps both heads' tiles alive at a
  transition). Filler work (remaining projections, W_o load, first half
  of the output projection) rides on non-transition units. Head order
  within a query block is [1, 0, 2, 3] so the final normalize is an odd
  head (its den needs no DMA hop off p64).
"""

import numpy as np

import concourse.bass as bass
import concourse.mybir as mybir
import concourse.tile as tile
from concourse import bacc
from concourse.bass_utils import run_bass_kernel_spmd

B, T, D, OUT, H = 2, 2048, 1024, 1024, 16
DO = 256            # output columns per core (4 heads x 64)
DEPTH = 64
NH = 4              # heads per core
KT = D // 128       # 8 contraction tiles for the projections
TT = T // 128       # 16 key tiles
NB = T // 512       # 4 query/time blocks
F32 = mybir.dt.float32
F16 = mybir.dt.float16
BF16 = mybir.dt.bfloat16
MMDT = BF16
EXP = mybir.ActivationFunctionType.Exp
MULT = mybir.AluOpType.mult
ADD = mybir.AluOpType.add

H_ORDER = [1, 0, 2, 3]   # emission order of heads within a query block

_CACHE = {}


def build_attention(nc, dbg=None):
    xt = nc.declare_dram_parameter("xt", [D, T], MMDT, isOutput=False)
    # weights are pre-arranged on the host into the SBUF layout so the
    # DMAs are contiguous full-rate transfers
    wq = nc.declare_dram_parameter("wq", [128, KT * DO], MMDT, isOutput=False)
    wk = nc.declare_dram_parameter("wk", [128, KT * DO], MMDT, isOutput=False)
    wv = nc.declare_dram_parameter("wv", [128, KT * DO], MMDT, isOutput=False)
    wo = nc.declare_dram_parameter("wo", [128, 2 * OUT], MMDT, isOutput=False)
    bq2 = nc.declare_dram_parameter("bq2", [128, 2], F32, isOutput=False)
    bv = nc.declare_dram_parameter("bv", [DO], F32, isOutput=False)
    bcol = nc.declare_dram_parameter("bcol", [128, TT], F32, isOutput=False)
    outT = nc.declare_dram_parameter("outT", [OUT, T], F16, isOutput=True)

    with tile.TileContext(nc) as tc:
        with (
            tc.tile_pool(name="cw", bufs=1) as cw,
            tc.tile_pool(name="stage", bufs=6) as stage,
            tc.tile_pool(name="persist", bufs=1) as persist,
            tc.tile_pool(name="small", bufs=2) as small,
            tc.tile_pool(name="ptp", bufs=6) as ptp,
            tc.tile_pool(name="px", bufs=1) as px,
            tc.tile_pool(name="ps_s", bufs=2, space="PSUM") as ps_s,
            tc.tile_pool(name="ps_mm", bufs=2, space="PSUM") as ps_mm,
        ):
            # ---- warmup: wake the PE HAM clock gate and the ACT exp table
            # while the input DMAs are in flight ----
            ones_f = cw.tile([128, NH], F32, tag="ones")
            nc.vector.memset(ones_f[:], 1.0)
            warm_w = cw.tile([128, 256], MMDT, tag="warmw")
            nc.vector.memset(warm_w[:], 0.0)
            warm_ps = ps_s.tile([128, 256], F32, tag="s", name="warm_ps")
            for _ in range(14):
                nc.tensor.matmul(warm_ps[:, :256], warm_w[:, :128], warm_w[:, :256],
                                 start=True, stop=True)
            warm_pt = cw.tile([128, NH], MMDT, tag="warmpt")
            nc.scalar.activation(warm_pt[:], ones_f[:], EXP, scale=1.0)

            # ---- inputs; x tiles split in column halves across two DMA
            # queues so the first projections start ~3us in ----
            def load_bf16(pool, dram_ap, shape, tag, eng=None):
                r = pool.tile(shape, MMDT, tag=tag, name=f"r_{tag}")
                (eng or nc.sync).dma_start(out=r[:], in_=dram_ap)
                return r

            wk_r = load_bf16(px, wk[:, :].rearrange("p (kt m) -> p kt m", m=DO), [128, KT, DO], "wk")
            wq_r = load_bf16(px, wq[:, :].rearrange("p (kt m) -> p kt m", m=DO), [128, KT, DO], "wq", eng=nc.scalar)
            xr = []
            for kt in range(KT):
                xr.append(load_bf16(px, xt[kt * 128:(kt + 1) * 128, :], [128, T], f"xr{kt}",
                                    eng=(nc.sync if kt % 2 == 0 else nc.scalar)))
            wv_r = load_bf16(px, wv[:, :].rearrange("p (kt m) -> p kt m", m=DO), [128, KT, DO], "wv")

            # ---- persistent activation tiles; kth zero-halves first on
            # gpsimd (they gate the first S stationary), then constants,
            # then the PV tile zeroing ----
            qt2 = [persist.tile([128, T], MMDT, tag=f"qt{mi}", name=f"qt{mi}") for mi in range(2)]
            kth = [persist.tile([128, T], MMDT, tag=f"kh{h}", name=f"kh{h}") for h in range(NH)]
            vp = persist.tile([128, TT, NH * 128], MMDT, tag="vp")
            at2p = [persist.tile([128, T], MMDT, tag=f"atp{p}", name=f"atp{p}") for p in range(2)]
            for h in H_ORDER:
                lo, hi = ((64, 128) if h % 2 == 0 else (0, 64))
                nc.gpsimd.memset(kth[h][lo:hi, :], 0.0)

            # ---- constants (gpsimd queue, off the x critical path) ----
            bq_sb = cw.tile([128, 2], F32, tag="bq")
            nc.gpsimd.dma_start(out=bq_sb[:], in_=bq2[:, :])
            bcol_sb = cw.tile([128, TT], F32, tag="bcol")
            nc.gpsimd.dma_start(out=bcol_sb[:], in_=bcol[:, :])
            bv_sb = cw.tile([128, DO], F32, tag="bv")
            bv_ap = bv.ap()
            bv_bcast = bass.AP(tensor=bv_ap.tensor, offset=bv_ap.offset, ap=[[0, 128], [1, DO]])
            nc.gpsimd.dma_start(out=bv_sb[:], in_=bv_bcast)

            # PV stationary: [kt, head, 128 cols]; per-head column placement
            # is parity-asymmetric (see module docstring)
            nc.gpsimd.memset(vp[:, 0:2, :], 0.0)
            nc.gpsimd.memset(vp[:, 2:TT, :], 0.0)
            wo_r = load_bf16(px, wo[:, :].rearrange("p (j n) -> p j n", j=2), [128, 2, OUT], "wo", eng=nc.gpsimd)

            # ---- emission helpers (advanced by the interleaver) ----
            def q_group(mi, nb, pool_tag="attn"):
                """One [128,512] Q projection psum group: 8 matmuls + biased copy."""
                pool = ps_s if pool_tag == "s" else ps_mm
                ps = pool.tile([128, 1024], F32, tag=pool_tag, name=f"ps_q{mi}_{nb}")
                for kt in range(KT):
                    nc.tensor.matmul(
                        ps[:, :512],
                        wq_r[:, kt, mi * 128:(mi + 1) * 128],
                        xr[kt][:, nb * 512:(nb + 1) * 512],
                        start=(kt == 0),
                        stop=(kt == KT - 1),
                    )
                nc.vector.tensor_scalar_add(
                    qt2[mi][:, nb * 512:(nb + 1) * 512], ps[:, :512], bq_sb[:, mi:mi + 1]
                )

            def k_group(mi, nb, pool_tag="attn"):
                """K projection group, split-written into the two kth tiles."""
                pool = ps_s if pool_tag == "s" else ps_mm
                ps = pool.tile([128, 1024], F32, tag=pool_tag, name=f"ps_k{mi}_{nb}")
                for kt in range(KT):
                    nc.tensor.matmul(
                        ps[:, :512],
                        wk_r[:, kt, mi * 128:(mi + 1) * 128],
                        xr[kt][:, nb * 512:(nb + 1) * 512],
                        start=(kt == 0),
                        stop=(kt == KT - 1),
                    )
                sl = slice(nb * 512, (nb + 1) * 512)
                nc.vector.tensor_scalar_add(
                    kth[2 * mi][0:64, sl], ps[0:64, :512], bq_sb[0:64, mi:mi + 1]
                )
                nc.vector.tensor_scalar_add(
                    kth[2 * mi + 1][64:128, sl], ps[64:128, :512], bq_sb[64:128, mi:mi + 1]
                )

            def v_group(tt):
                ps = ps_mm.tile([128, 1024], F32, tag="attn", name=f"ps_v{tt}")
                for kt in range(KT):
                    nc.tensor.matmul(
                        ps[:, :DO],
                        xr[kt][:, tt * 128:(tt + 1) * 128],
                        wv_r[:, kt, :],
                        start=(kt == 0),
                        stop=(kt == KT - 1),
                    )
                vpt = vp[:, tt, :].rearrange("p (h c) -> p h c", c=128)
                # even heads (0,2): V at cols 0:64; odd heads (1,3): cols 64:128
                nc.vector.tensor_tensor(
                    vpt[:, 0::2, 0:64],
                    ps[:, :DO].rearrange("p (h c) -> p h c", c=64)[:, 0::2, :],
                    bv_sb[:, :].rearrange("p (h c) -> p h c", c=64)[:, 0::2, :],
                    ADD,
                )
                nc.vector.tensor_tensor(
                    vpt[:, 1::2, 64:128],
                    ps[:, :DO].rearrange("p (h c) -> p h c", c=64)[:, 1::2, :],
                    bv_sb[:, :].rearrange("p (h c) -> p h c", c=64)[:, 1::2, :],
                    ADD,
                )
                nc.gpsimd.tensor_copy(out=vpt[:, 0::2, 64:65], in_=ones_f[:, 0:2, None])
                nc.gpsimd.tensor_copy(out=vpt[:, 1::2, 0:1], in_=ones_f[:, 2:4, None])

            def emit_pv(h, attn_ps, kt, pt):
                for half in range(2):
                    nc.tensor.matmul(
                        attn_ps[:, half * 512:(half + 1) * 512],
                        vp[:, kt, h * 128:(h + 1) * 128],
                        pt[:, half * 512:(half + 1) * 512],
                        start=(kt == 0),
                        stop=(kt == TT - 1),
                    )

            def normalize(qbp, h, attn_ps, split=False):
                sl = slice(qbp * 1024, (qbp + 1) * 1024)
                if h % 2 == 0:
                    # den at psum p64; move to p0 for recip+broadcast
                    den = cw.tile([65, 1024], F32, tag="den", name=f"den{qbp}_{h}")
                    nc.vector.tensor_copy(out=den[64:65, :], in_=attn_ps[64:65, :])
                    d0 = cw.tile([1, 1024], F32, tag="d0", name=f"d0{qbp}_{h}")
                    nc.sync.dma_start(out=d0[:], in_=den[64:65, :])
                    rec = cw.tile([1, 1024], F32, tag="rec", name=f"rec{qbp}_{h}")
                    nc.vector.reciprocal_approx_fast(rec[:], d0[:])
                    rb = small.tile([64, 1024], F32, tag="rb", name=f"rb{qbp}_{h}")
                    nc.gpsimd.partition_broadcast(rb[:], rec[:])
                    nc.vector.tensor_tensor(
                        at2p[h // 2][0:64, sl], attn_ps[0:64, :], rb[:], MULT
                    )
                else:
                    # den already at psum p0: recip straight off psum.
                    # split=True pipelines the two 512-column halves so the
                    # tail output projection can start ~2.5us sooner.
                    rbh = small.tile([128, 1024], F32, tag="rbh", name=f"rbh{qbp}_{h}")
                    halves = ((0, 1024),) if not split else ((0, 512), (512, 1024))
                    for lo, hi in halves:
                        rec = cw.tile([1, 1024], F32, tag="rec", name=f"rec{qbp}_{h}_{lo}")
                        nc.vector.reciprocal_approx_fast(rec[:, 0:hi - lo], attn_ps[0:1, lo:hi])
                        nc.gpsimd.partition_broadcast(rbh[:, lo:hi], rec[:, 0:hi - lo])
                        nc.vector.tensor_tensor(
                            at2p[h // 2][64:128, qbp * 1024 + lo:qbp * 1024 + hi],
                            attn_ps[64:128, lo:hi], rbh[64:128, lo:hi], MULT
                        )

            def c_group(nt, tb, evac=None):
                ps = ps_mm.tile([128, 1024], F32, tag="attn", name=f"ps_c{nt}_{tb}")
                for j in range(2):
                    nc.tensor.matmul(
                        ps[:, :512],
                        wo_r[:, j, nt * 128:(nt + 1) * 128],
                        at2p[j][:, tb * 512:(tb + 1) * 512],
                        start=(j == 0),
                        stop=(j == 1),
                    )
                o_sb = stage.tile([128, 512], F16, tag="stage", name="o_sb")
                if evac == "s":
                    nc.scalar.copy(o_sb[:], ps[:, :512])
                else:
                    nc.vector.tensor_copy(out=o_sb[:], in_=ps[:, :512])
                nc.sync.dma_start(
                    out=outT[nt * 128:(nt + 1) * 128, tb * 512:(tb + 1) * 512],
                    in_=o_sb[:],
                )

            # ---- emission schedule ----
            # minimal upfront work for the first unit, then ONE globally
            # software-pipelined stream over all 128 attention units.
            k_group(0, 0, pool_tag="s")
            q_group(0, 0, pool_tag="s")
            q_group(0, 1, pool_tag="s")

            # v_group(tt) must be emitted >=1 unit before its PV consumer
            # (PV for kt=tt is emitted at idx tt+1); K(0,nb) before idx 4nb.
            era1 = [
                [lambda: v_group(0), lambda: v_group(1), lambda: v_group(2)],
                [lambda: v_group(3), lambda: k_group(0, 1)],
                [lambda: v_group(4)], [lambda: v_group(5)], [lambda: v_group(6)],
                [lambda: v_group(7), lambda: k_group(0, 2)],
                [lambda: v_group(8)], [lambda: v_group(9)], [lambda: v_group(10)],
                [lambda: v_group(11), lambda: k_group(0, 3)],
                [lambda: v_group(12)], [lambda: v_group(13)],
                [lambda: v_group(14)], [lambda: v_group(15)],
                None, None,
            ]
            era2 = [
                None,
                [lambda: k_group(1, 0)], [lambda: k_group(1, 1)],
                [lambda: k_group(1, 2)], [lambda: k_group(1, 3)],
                [lambda: q_group(1, 0)], [lambda: q_group(1, 1)],
            ] + [None] * 9
            era3 = [None] * 32
            era3[2] = [lambda: q_group(0, 2)]
            era3[6] = [lambda: q_group(0, 3)]
            era3[10] = [lambda: q_group(1, 2)]
            era3[14] = [lambda: q_group(1, 3)]
            c_work = [(nt, tb) for tb in range(2) for nt in range(OUT // 128)]

            units = [(qbp, h, kt) for qbp in range(2) for h in H_ORDER for kt in range(TT)]
            attn_tiles = {}
            prev = None
            for idx, (qbp, h, kt) in enumerate(units):
                if kt == 0:
                    attn_tiles[(qbp, h)] = ps_mm.tile(
                        [128, 1024], F32, tag="attn", name=f"attn_{qbp}_{h}"
                    )
                s_ps = ps_s.tile([128, 1024], F32, tag="s", name=f"s_{qbp}_{h}_{kt}")
                for half in range(2):
                    nc.tensor.matmul(
                        s_ps[:, half * 512:(half + 1) * 512],
                        kth[h][:, kt * 128:(kt + 1) * 128],
                        qt2[h // 2][:, qbp * 1024 + half * 512:qbp * 1024 + (half + 1) * 512],
                        start=True,
                        stop=True,
                    )
                pt = ptp.tile([128, 1024], MMDT, tag="pt")
                nc.scalar.activation(
                    pt[:], s_ps[:], EXP, bias=bcol_sb[:, kt:kt + 1], scale=0.125
                )
                if prev is not None:
                    pq, ph, pk, ppt = prev
                    emit_pv(ph, attn_tiles[(pq, ph)], pk, ppt)
                    if pk == TT - 1:
                        normalize(pq, ph, attn_tiles.pop((pq, ph)))
                # filler work, away from head-transition units
                if idx < 16:
                    for item in era1[idx] or []:
                        item()
                elif idx < 32:
                    for item in era2[idx - 16] or []:
                        item()
                elif idx < 64:
                    for item in era3[idx - 32] or []:
                        item()
                elif idx >= 64 and 1 <= kt <= 14 and kt % 3 == 1 and c_work:
                    nt, tb = c_work.pop(0)
                    c_group(nt, tb)
                prev = (qbp, h, kt, pt)
            pq, ph, pk, ppt = prev
            emit_pv(ph, attn_tiles[(pq, ph)], pk, ppt)
            normalize(pq, ph, attn_tiles.pop((pq, ph)), split=True)

            while c_work:
                nt, tb = c_work.pop(0)
                c_group(nt, tb)
            # tail: ScalarE is idle now -- alternate psum evacuation between
            # the vector and scalar engines so the matmul stream never waits
            for i, (nt, tb) in enumerate([(nt, tb) for tb in range(2, NB) for nt in range(OUT // 128)]):
                c_group(nt, tb, evac=("s" if i % 2 == 0 else "v"))

            if dbg:
                for mi in range(2):
                    nc.sync.dma_start(out=dbg["d_qt"][mi][:, :], in_=qt2[mi][:])
                for h in range(NH):
                    nc.sync.dma_start(out=dbg["d_kt"][h][:, :], in_=kth[h][:])
                for j in range(2):
                    nc.sync.dma_start(out=dbg["d_at"][j][:, :], in_=at2p[j][:])
                nc.sync.dma_start(out=dbg["d_vp"][:, :, :], in_=vp[:])


def _build():
    nc = bacc.Bacc(trn_type="TRN2")
    build_attention(nc)
    nc.compile()
    return nc


def _get_nc():
    if "nc" not in _CACHE:
        _CACHE["nc"] = _build()
    return _CACHE["nc"]


def make_in_maps(x, W_q, b_q, W_k, W_v, b_v, W_o, bias):
    import ml_dtypes
    bf16 = ml_dtypes.bfloat16

    def warr(w):
        # [D, DO] -> SBUF layout [128, KT*DO] (partition-major, kt-tiled)
        return np.ascontiguousarray(
            w.reshape(KT, 128, DO).transpose(1, 0, 2).reshape(128, KT * DO))

    def woarr(w):
        # [2*128, OUT] -> [two*64+p, j, n] -> [128, 2*OUT]
        return np.ascontiguousarray(
            w.reshape(2, 2, 64, OUT).transpose(1, 2, 0, 3).reshape(128, 2 * OUT))

    in_maps = []
    xtb = [np.ascontiguousarray(x[b].T.astype(bf16)) for b in range(B)]
    wqb = W_q.astype(bf16)
    wkb = W_k.astype(bf16)
    wvb = W_v.astype(bf16)
    wob = W_o.astype(bf16)
    for c in range(8):
        b, hg = divmod(c, 4)
        sl = slice(hg * DO, (hg + 1) * DO)
        in_maps.append({
            "xt": xtb[b],
            "wq": warr(wqb[:, sl]),
            "wk": warr(wkb[:, sl]),
            "wv": warr(wvb[:, sl]),
            "wo": woarr(wob[sl, :]),
            "bq2": np.ascontiguousarray(b_q[sl].reshape(2, 128).T),
            "bv": np.ascontiguousarray(b_v[sl]),
            "bcol": np.ascontiguousarray(bias.reshape(TT, 128).T),
        })
    return in_maps


def kernel(x, W_q, b_q, W_k, b_k, W_v, b_v, W_o, b_o, bias, **_ignored):
    x = np.asarray(x, dtype=np.float32)
    W_q = np.asarray(W_q, dtype=np.float32)
    W_k = np.asarray(W_k, dtype=np.float32)
    W_v = np.asarray(W_v, dtype=np.float32)
    W_o = np.asarray(W_o, dtype=np.float32)
    b_q = np.asarray(b_q, dtype=np.float32)
    b_v = np.asarray(b_v, dtype=np.float32)
    b_o = np.asarray(b_o, dtype=np.float32)
    bias = np.asarray(bias, dtype=np.float32)

    nc = _get_nc()
    in_maps = make_in_maps(x, W_q, b_q, W_k, W_v, b_v, W_o, bias)
    _CACHE["in_maps"] = in_maps
    res = run_bass_kernel_spmd(nc, in_maps, list(range(8)))
    out = np.zeros((B, T, OUT), dtype=np.float32)
    for c in range(8):
        out[c // 4] += res.results[c]["outT"].T.astype(np.float32)
    out += b_o
    return out


# revision 23
# speedup vs baseline: 1.0109x; 1.0109x over previous
"""Multi-head attention (B=2, T=2048, D=OUT=1024, H=16) on 8 TRN2 NeuronCores.

Sharding: data-parallel over batch (2 groups) x tensor-parallel over heads
(4 groups of 4 heads). Core c handles batch c//4, heads (c%4)*4..(c%4)*4+4.
Each core computes Q^T/K^T/V for its head group, streams softmax(QK^T)V
in transposed layout (keys on partitions), and a partial output projection
through its W_o row block. The host sums the 4 partials per batch and adds
b_o.

Device-side layout notes:
- x is fed transposed ([D, T]) so Q^T/K^T come straight out of the PE.
- The K projection psum is split-written directly into the per-head
  zero-padded kth tiles (no staging copy): head 2mi keeps psum rows 0:64,
  head 2mi+1 keeps rows 64:128, matching where the paired Q^T tile holds
  that head's rows. Every attention matmul then contracts K=128 (K=64
  matmuls do not register as PE activity for the HAM clock gate and run
  at half clock forever).
- The PV stationary is a full 128-column tile per (kt, head) so LDWEIGHTS
  gets fast-weight-load; the softmax denominator rides along as a ones
  column inside it. Per-head column placement is parity-asymmetric so the
  PV output lands directly at the at2p partitions that head occupies:
    even h: V at cols 0:64 (out rows 0:64), ones at col 64 (den at p64)
    odd  h: ones at col 0 (den at p0), V at cols 64:128 (out rows 64:128)
  Unused columns are zeroed once at startup. The odd-head path needs no
  cross-partition DMA hop at all: reciprocal reads the den row straight
  from psum partition 0 and partition_broadcast fills p64:128.
- Matmul operands are bf16 (fp32 PSUM accumulation): full PE clock and
  fast weight load; fp32r measured 2x slower.
- Output partials are written as fp16 ([OUT, T]) -- halves the outbound
  DMA; the host accumulates in fp32 (adds ~1e-4 relative error).
- Emission is one globally software-pipelined stream over all 128
  attention units (PV of unit u-1 emitted inside unit u, across head
  boundaries; attn psum bufs=2 keeps both heads' tiles alive at a
  transition). Filler work (remaining projections, W_o load, first half
  of the output projection) rides on non-transition units. Head order
  within a query block is [1, 0, 2, 3] so the final normalize is an odd
  head (its den needs no DMA hop off p64).
"""

import numpy as np

import concourse.bass as bass
import concourse.mybir as mybir
import concourse.tile as tile
from concourse import bacc
from concourse.bass_utils import run_bass_kernel_spmd

B, T, D, OUT, H = 2, 2048, 1024, 1024, 16
DO = 256            # output columns per core (4 heads x 64)
DEPTH = 64
NH = 4              # heads per core
KT = D // 128       # 8 contraction tiles for the projections
TT = T // 128       # 16 key tiles
NB = T // 512       # 4 query/time blocks
F32 = mybir.dt.float32
F16 = mybir.dt.float16
BF16 = mybir.dt.bfloat16
MMDT = BF16
EXP = mybir.ActivationFunctionType.Exp
MULT = mybir.AluOpType.mult
ADD = mybir.AluOpType.add

H_ORDER = [1, 0, 2, 3]   # emission order of heads within a query block

_CACHE = {}


def build_attention(nc, dbg=None):
    xt = nc.declare_dram_parameter("xt", [D, T], MMDT, isOutput=False)
    # weights are pre-arranged on the host into the SBUF layout so the
    # DMAs are contiguous full-rate transfers
    wq = nc.declare_dram_parameter("wq", [128, KT * DO], MMDT, isOutput=False)
    wk = nc.declare_dram_parameter("wk", [128, KT * DO], MMDT, isOutput=False)
    wv = nc.declare_dram_parameter("wv", [128, KT * DO], MMDT, isOutput=False)
    wo = nc.declare_dram_parameter("wo", [128, 2 * OUT], MMDT, isOutput=False)
    bq2 = nc.declare_dram_parameter("bq2", [128, 2], F32, isOutput=False)
    bv = nc.declare_dram_parameter("bv", [DO], F32, isOutput=False)
    bcol = nc.declare_dram_parameter("bcol", [128, TT], F32, isOutput=False)
    outT = nc.declare_dram_parameter("outT", [OUT, T], F16, isOutput=True)

    with tile.TileContext(nc) as tc:
        with (
            tc.tile_pool(name="cw", bufs=1) as cw,
            tc.tile_pool(name="stage", bufs=6) as stage,
            tc.tile_pool(name="persist", bufs=1) as persist,
            tc.tile_pool(name="small", bufs=2) as small,
            tc.tile_pool(name="ptp", bufs=6) as ptp,
            tc.tile_pool(name="px", bufs=1) as px,
            tc.tile_pool(name="ps_s", bufs=2, space="PSUM") as ps_s,
            tc.tile_pool(name="ps_mm", bufs=2, space="PSUM") as ps_mm,
        ):
            # ---- warmup: wake the PE HAM clock gate and the ACT exp table
            # while the input DMAs are in flight ----
            ones_f = cw.tile([128, NH], F32, tag="ones")
            nc.vector.memset(ones_f[:], 1.0)
            warm_w = cw.tile([128, 256], MMDT, tag="warmw")
            nc.vector.memset(warm_w[:], 0.0)
            warm_ps = ps_s.tile([128, 256], F32, tag="s", name="warm_ps")
            for _ in range(28):
                nc.tensor.matmul(warm_ps[:, :256], warm_w[:, :128], warm_w[:, :256],
                                 start=True, stop=True)
            warm_pt = cw.tile([128, NH], MMDT, tag="warmpt")
            nc.scalar.activation(warm_pt[:], ones_f[:], EXP, scale=1.0)

            # ---- inputs; x tiles split in column halves across two DMA
            # queues so the first projections start ~3us in ----
            def load_bf16(pool, dram_ap, shape, tag, eng=None):
                r = pool.tile(shape, MMDT, tag=tag, name=f"r_{tag}")
                (eng or nc.sync).dma_start(out=r[:], in_=dram_ap)
                return r

            wk_r = load_bf16(px, wk[:, :].rearrange("p (kt m) -> p kt m", m=DO), [128, KT, DO], "wk")
            wq_r = load_bf16(px, wq[:, :].rearrange("p (kt m) -> p kt m", m=DO), [128, KT, DO], "wq", eng=nc.scalar)
            # x tiles arrive in 512-column chunks, nb-major, split across
            # two DMA queues, so the first projection chains finish while
            # the bulk of x is still in flight
            xr = [px.tile([128, T], MMDT, tag=f"xr{kt}", name=f"r_xr{kt}") for kt in range(KT)]
            for nb in range(NB):
                csl = slice(nb * 512, (nb + 1) * 512)
                for kt in range(KT):
                    eng = nc.sync if kt % 2 == 0 else nc.scalar
                    eng.dma_start(out=xr[kt][:, csl], in_=xt[kt * 128:(kt + 1) * 128, csl])

            # ---- gpsimd queue: biases first (they gate the projection
            # copies), kth zero-halves (gate the first S stationary), wv,
            # then PV-tile zeroing and wo ----
            qt2 = [persist.tile([128, T], MMDT, tag=f"qt{mi}", name=f"qt{mi}") for mi in range(2)]
            kth = [persist.tile([128, T], MMDT, tag=f"kh{h}", name=f"kh{h}") for h in range(NH)]
            vp = persist.tile([128, TT, NH * 128], MMDT, tag="vp")
            at2p = [persist.tile([128, T], MMDT, tag=f"atp{p}", name=f"atp{p}") for p in range(2)]
            bq_sb = cw.tile([128, 2], F32, tag="bq")
            nc.gpsimd.dma_start(out=bq_sb[:], in_=bq2[:, :])
            for h in H_ORDER:
                lo, hi = ((64, 128) if h % 2 == 0 else (0, 64))
                nc.gpsimd.memset(kth[h][lo:hi, :], 0.0)
            wv_r = load_bf16(px, wv[:, :].rearrange("p (kt m) -> p kt m", m=DO), [128, KT, DO], "wv", eng=nc.gpsimd)
            bcol_sb = cw.tile([128, TT], F32, tag="bcol")
            nc.gpsimd.dma_start(out=bcol_sb[:], in_=bcol[:, :])
            bv_sb = cw.tile([128, DO], F32, tag="bv")
            bv_ap = bv.ap()
            bv_bcast = bass.AP(tensor=bv_ap.tensor, offset=bv_ap.offset, ap=[[0, 128], [1, DO]])
            nc.gpsimd.dma_start(out=bv_sb[:], in_=bv_bcast)
            # PV stationary: [kt, head, 128 cols]; per-head column placement
            # is parity-asymmetric (see module docstring)
            nc.gpsimd.memset(vp[:, 0:4, :], 0.0)
            nc.gpsimd.memset(vp[:, 4:TT, :], 0.0)
            wo_r = load_bf16(px, wo[:, :].rearrange("p (j n) -> p j n", j=2), [128, 2, OUT], "wo", eng=nc.gpsimd)

            # ---- emission helpers (advanced by the interleaver) ----
            def q_group(mi, nb, pool_tag="attn"):
                """One [128,512] Q projection psum group: 8 matmuls + biased copy."""
                pool = ps_s if pool_tag == "s" else ps_mm
                ps = pool.tile([128, 1024], F32, tag=pool_tag, name=f"ps_q{mi}_{nb}")
                for kt in range(KT):
                    nc.tensor.matmul(
                        ps[:, :512],
                        wq_r[:, kt, mi * 128:(mi + 1) * 128],
                        xr[kt][:, nb * 512:(nb + 1) * 512],
                        start=(kt == 0),
                        stop=(kt == KT - 1),
                    )
                nc.vector.tensor_scalar_add(
                    qt2[mi][:, nb * 512:(nb + 1) * 512], ps[:, :512], bq_sb[:, mi:mi + 1]
                )

            def k_group(mi, nb, pool_tag="attn"):
                """K projection group, split-written into the two kth tiles."""
                pool = ps_s if pool_tag == "s" else ps_mm
                ps = pool.tile([128, 1024], F32, tag=pool_tag, name=f"ps_k{mi}_{nb}")
                for kt in range(KT):
                    nc.tensor.matmul(
                        ps[:, :512],
                        wk_r[:, kt, mi * 128:(mi + 1) * 128],
                        xr[kt][:, nb * 512:(nb + 1) * 512],
                        start=(kt == 0),
                        stop=(kt == KT - 1),
                    )
                sl = slice(nb * 512, (nb + 1) * 512)
                nc.vector.tensor_scalar_add(
                    kth[2 * mi][0:64, sl], ps[0:64, :512], bq_sb[0:64, mi:mi + 1]
                )
                nc.vector.tensor_scalar_add(
                    kth[2 * mi + 1][64:128, sl], ps[64:128, :512], bq_sb[64:128, mi:mi + 1]
                )

            def v_group(tt):
                ps = ps_mm.tile([128, 1024], F32, tag="attn", name=f"ps_v{tt}")
                for kt in range(KT):
                    nc.tensor.matmul(
                        ps[:, :DO],
                        xr[kt][:, tt * 128:(tt + 1) * 128],
                        wv_r[:, kt, :],
                        start=(kt == 0),
                        stop=(kt == KT - 1),
                    )
                vpt = vp[:, tt, :].rearrange("p (h c) -> p h c", c=128)
                # even heads (0,2): V at cols 0:64; odd heads (1,3): cols 64:128
                nc.vector.tensor_tensor(
                    vpt[:, 0::2, 0:64],
                    ps[:, :DO].rearrange("p (h c) -> p h c", c=64)[:, 0::2, :],
                    bv_sb[:, :].rearrange("p (h c) -> p h c", c=64)[:, 0::2, :],
                    ADD,
                )
                nc.vector.tensor_tensor(
                    vpt[:, 1::2, 64:128],
                    ps[:, :DO].rearrange("p (h c) -> p h c", c=64)[:, 1::2, :],
                    bv_sb[:, :].rearrange("p (h c) -> p h c", c=64)[:, 1::2, :],
                    ADD,
                )
                nc.gpsimd.tensor_copy(out=vpt[:, 0::2, 64:65], in_=ones_f[:, 0:2, None])
                nc.gpsimd.tensor_copy(out=vpt[:, 1::2, 0:1], in_=ones_f[:, 2:4, None])

            def emit_pv(h, attn_ps, kt, pt):
                for half in range(2):
                    nc.tensor.matmul(
                        attn_ps[:, half * 512:(half + 1) * 512],
                        vp[:, kt, h * 128:(h + 1) * 128],
                        pt[:, half * 512:(half + 1) * 512],
                        start=(kt == 0),
                        stop=(kt == TT - 1),
                    )

            def normalize(qbp, h, attn_ps, split=False):
                sl = slice(qbp * 1024, (qbp + 1) * 1024)
                if h % 2 == 0:
                    # den at psum p64; move to p0 for recip+broadcast
                    den = cw.tile([65, 1024], F32, tag="den", name=f"den{qbp}_{h}")
                    nc.vector.tensor_copy(out=den[64:65, :], in_=attn_ps[64:65, :])
                    d0 = cw.tile([1, 1024], F32, tag="d0", name=f"d0{qbp}_{h}")
                    nc.sync.dma_start(out=d0[:], in_=den[64:65, :])
                    rec = cw.tile([1, 1024], F32, tag="rec", name=f"rec{qbp}_{h}")
                    nc.vector.reciprocal_approx_fast(rec[:], d0[:])
                    rb = small.tile([64, 1024], F32, tag="rb", name=f"rb{qbp}_{h}")
                    nc.gpsimd.partition_broadcast(rb[:], rec[:])
                    nc.vector.tensor_tensor(
                        at2p[h // 2][0:64, sl], attn_ps[0:64, :], rb[:], MULT
                    )
                else:
                    # den already at psum p0: recip straight off psum.
                    # split=True pipelines the two 512-column halves so the
                    # tail output projection can start ~2.5us sooner.
                    rbh = small.tile([128, 1024], F32, tag="rbh", name=f"rbh{qbp}_{h}")
                    halves = ((0, 1024),) if not split else ((0, 512), (512, 1024))
                    for lo, hi in halves:
                        rec = cw.tile([1, 1024], F32, tag="rec", name=f"rec{qbp}_{h}_{lo}")
                        nc.vector.reciprocal_approx_fast(rec[:, 0:hi - lo], attn_ps[0:1, lo:hi])
                        nc.gpsimd.partition_broadcast(rbh[:, lo:hi], rec[:, 0:hi - lo])
                        nc.vector.tensor_tensor(
                            at2p[h // 2][64:128, qbp * 1024 + lo:qbp * 1024 + hi],
                            attn_ps[64:128, lo:hi], rbh[64:128, lo:hi], MULT
                        )

            def c_group(nt, tb, evac=None):
                ps = ps_mm.tile([128, 1024], F32, tag="attn", name=f"ps_c{nt}_{tb}")
                for j in range(2):
                    nc.tensor.matmul(
                        ps[:, :512],
                        wo_r[:, j, nt * 128:(nt + 1) * 128],
                        at2p[j][:, tb * 512:(tb + 1) * 512],
                        start=(j == 0),
                        stop=(j == 1),
                    )
                o_sb = stage.tile([128, 512], F16, tag="stage", name="o_sb")
                if evac == "s":
                    nc.scalar.copy(o_sb[:], ps[:, :512])
                else:
                    nc.vector.tensor_copy(out=o_sb[:], in_=ps[:, :512])
                nc.sync.dma_start(
                    out=outT[nt * 128:(nt + 1) * 128, tb * 512:(tb + 1) * 512],
                    in_=o_sb[:],
                )

            # ---- emission schedule ----
            # minimal upfront work for the first unit, then ONE globally
            # software-pipelined stream over all 128 attention units.
            # prep: everything the first S unit needs, plus V groups whose
            # x columns arrive early (they run inside the x DMA wait)
            k_group(0, 0, pool_tag="s")
            q_group(0, 0, pool_tag="s")
            q_group(0, 1, pool_tag="s")
            for tt in range(4):
                v_group(tt)

            # v_group(tt) must be emitted >=1 unit before its PV consumer
            # (PV for kt=tt is emitted at idx tt+1); K(0,nb) before idx 4nb.
            era1 = [
                [lambda: v_group(4)], [lambda: v_group(5)],
                [lambda: k_group(0, 1)],
                [lambda: v_group(6)], [lambda: v_group(7)], [lambda: v_group(8)],
                [lambda: k_group(0, 2)],
                [lambda: v_group(9)], [lambda: v_group(10)], [lambda: v_group(11)],
                [lambda: k_group(0, 3)],
                [lambda: v_group(12)], [lambda: v_group(13)],
                [lambda: v_group(14)], [lambda: v_group(15)],
                None,
            ]
            era2 = [None] * 16
            era2[1] = [lambda: k_group(1, 0)]
            era2[3] = [lambda: k_group(1, 1)]
            era2[5] = [lambda: k_group(1, 2)]
            era2[7] = [lambda: k_group(1, 3)]
            era2[9] = [lambda: q_group(1, 0)]
            era2[11] = [lambda: q_group(1, 1)]
            era3 = [None] * 32
            era3[2] = [lambda: q_group(0, 2)]
            era3[6] = [lambda: q_group(0, 3)]
            era3[10] = [lambda: q_group(1, 2)]
            era3[14] = [lambda: q_group(1, 3)]
            c_work = [(nt, tb) for tb in range(2) for nt in range(OUT // 128)]

            units = [(qbp, h, kt) for qbp in range(2) for h in H_ORDER for kt in range(TT)]
            attn_tiles = {}
            prev = None
            for idx, (qbp, h, kt) in enumerate(units):
                if kt == 0:
                    attn_tiles[(qbp, h)] = ps_mm.tile(
                        [128, 1024], F32, tag="attn", name=f"attn_{qbp}_{h}"
                    )
                s_ps = ps_s.tile([128, 1024], F32, tag="s", name=f"s_{qbp}_{h}_{kt}")
                for half in range(2):
                    nc.tensor.matmul(
                        s_ps[:, half * 512:(half + 1) * 512],
                        kth[h][:, kt * 128:(kt + 1) * 128],
                        qt2[h // 2][:, qbp * 1024 + half * 512:qbp * 1024 + (half + 1) * 512],
                        start=True,
                        stop=True,
                    )
                pt = ptp.tile([128, 1024], MMDT, tag="pt")
                nc.scalar.activation(
                    pt[:], s_ps[:], EXP, bias=bcol_sb[:, kt:kt + 1], scale=0.125
                )
                if prev is not None:
                    pq, ph, pk, ppt = prev
                    emit_pv(ph, attn_tiles[(pq, ph)], pk, ppt)
                    if pk == TT - 1:
                        normalize(pq, ph, attn_tiles.pop((pq, ph)))
                # filler work, away from head-transition units
                if idx < 16:
                    for item in era1[idx] or []:
                        item()
                elif idx < 32:
                    for item in era2[idx - 16] or []:
                        item()
                elif idx < 64:
                    for item in era3[idx - 32] or []:
                        item()
                elif idx >= 64 and 1 <= kt <= 14 and kt % 3 == 1 and c_work:
                    nt, tb = c_work.pop(0)
                    c_group(nt, tb)
                prev = (qbp, h, kt, pt)
            pq, ph, pk, ppt = prev
            emit_pv(ph, attn_tiles[(pq, ph)], pk, ppt)
            normalize(pq, ph, attn_tiles.pop((pq, ph)), split=True)

            while c_work:
                nt, tb = c_work.pop(0)
                c_group(nt, tb)
            # tail: ScalarE is idle now -- alternate psum evacuation between
            # the vector and scalar engines so the matmul stream never waits
            for i, (nt, tb) in enumerate([(nt, tb) for tb in range(2, NB) for nt in range(OUT // 128)]):
                c_group(nt, tb, evac=("s" if i % 2 == 0 else "v"))

            if dbg:
                for mi in range(2):
                    nc.sync.dma_start(out=dbg["d_qt"][mi][:, :], in_=qt2[mi][:])
                for h in range(NH):
                    nc.sync.dma_start(out=dbg["d_kt"][h][:, :], in_=kth[h][:])
                for j in range(2):
                    nc.sync.dma_start(out=dbg["d_at"][j][:, :], in_=at2p[j][:])
                nc.sync.dma_start(out=dbg["d_vp"][:, :, :], in_=vp[:])


def _build():
    nc = bacc.Bacc(trn_type="TRN2")
    build_attention(nc)
    nc.compile()
    return nc


def _get_nc():
    if "nc" not in _CACHE:
        _CACHE["nc"] = _build()
    return _CACHE["nc"]


def make_in_maps(x, W_q, b_q, W_k, W_v, b_v, W_o, bias):
    import ml_dtypes
    bf16 = ml_dtypes.bfloat16

    def warr(w):
        # [D, DO] -> SBUF layout [128, KT*DO] (partition-major, kt-tiled)
        return np.ascontiguousarray(
            w.reshape(KT, 128, DO).transpose(1, 0, 2).reshape(128, KT * DO))

    def woarr(w):
        # [2*128, OUT] -> [two*64+p, j, n] -> [128, 2*OUT]
        return np.ascontiguousarray(
            w.reshape(2, 2, 64, OUT).transpose(1, 2, 0, 3).reshape(128, 2 * OUT))

    in_maps = []
    xtb = [np.ascontiguousarray(x[b].T.astype(bf16)) for b in range(B)]
    wqb = W_q.astype(bf16)
    wkb = W_k.astype(bf16)
    wvb = W_v.astype(bf16)
    wob = W_o.astype(bf16)
    for c in range(8):
        b, hg = divmod(c, 4)
        sl = slice(hg * DO, (hg + 1) * DO)
        in_maps.append({
            "xt": xtb[b],
            "wq": warr(wqb[:, sl]),
            "wk": warr(wkb[:, sl]),
            "wv": warr(wvb[:, sl]),
            "wo": woarr(wob[sl, :]),
            "bq2": np.ascontiguousarray(b_q[sl].reshape(2, 128).T),
            "bv": np.ascontiguousarray(b_v[sl]),
            "bcol": np.ascontiguousarray(bias.reshape(TT, 128).T),
        })
    return in_maps


def kernel(x, W_q, b_q, W_k, b_k, W_v, b_v, W_o, b_o, bias, **_ignored):
    x = np.asarray(x, dtype=np.float32)
    W_q = np.asarray(W_q, dtype=np.float32)
    W_k = np.asarray(W_k, dtype=np.float32)
    W_v = np.asarray(W_v, dtype=np.float32)
    W_o = np.asarray(W_o, dtype=np.float32)
    b_q = np.asarray(b_q, dtype=np.float32)
    b_v = np.asarray(b_v, dtype=np.float32)
    b_o = np.asarray(b_o, dtype=np.float32)
    bias = np.asarray(bias, dtype=np.float32)

    nc = _get_nc()
    in_maps = make_in_maps(x, W_q, b_q, W_k, W_v, b_v, W_o, bias)
    _CACHE["in_maps"] = in_maps
    res = run_bass_kernel_spmd(nc, in_maps, list(range(8)))
    out = np.zeros((B, T, OUT), dtype=np.float32)
    for c in range(8):
        out[c // 4] += res.results[c]["outT"].T.astype(np.float32)
    out += b_o
    return out


# revision 24
# speedup vs baseline: 1.0323x; 1.0211x over previous
"""Multi-head attention (B=2, T=2048, D=OUT=1024, H=16) on 8 TRN2 NeuronCores.

Sharding: data-parallel over batch (2 groups) x tensor-parallel over heads
(4 groups of 4 heads). Core c handles batch c//4, heads (c%4)*4..(c%4)*4+4.
Each core computes Q^T/K^T/V for its head group, streams softmax(QK^T)V
in transposed layout (keys on partitions), and a partial output projection
through its W_o row block. The host sums the 4 partials per batch and adds
b_o.

Device-side layout notes:
- x is fed transposed ([D, T]) so Q^T/K^T come straight out of the PE.
- The K projection psum is split-written directly into the per-head
  zero-padded kth tiles (no staging copy): head 2mi keeps psum rows 0:64,
  head 2mi+1 keeps rows 64:128, matching where the paired Q^T tile holds
  that head's rows. Every attention matmul then contracts K=128 (K=64
  matmuls do not register as PE activity for the HAM clock gate and run
  at half clock forever).
- The PV stationary is a full 128-column tile per (kt, head) so LDWEIGHTS
  gets fast-weight-load; the softmax denominator rides along as a ones
  column inside it. Per-head column placement is parity-asymmetric so the
  PV output lands directly at the at2p partitions that head occupies:
    even h: V at cols 0:64 (out rows 0:64), ones at col 64 (den at p64)
    odd  h: ones at col 0 (den at p0), V at cols 64:128 (out rows 64:128)
  Unused columns are zeroed once at startup. The odd-head path needs no
  cross-partition DMA hop at all: reciprocal reads the den row straight
  from psum partition 0 and partition_broadcast fills p64:128.
- Matmul operands are bf16 (fp32 PSUM accumulation): full PE clock and
  fast weight load; fp32r measured 2x slower.
- Output partials are written as fp16 ([OUT, T]) -- halves the outbound
  DMA; the host accumulates in fp32 (adds ~1e-4 relative error).
- Emission is one globally software-pipelined stream over all 128
  attention units (PV of unit u-1 emitted inside unit u, across head
  boundaries; attn psum bufs=2 keeps both heads' tiles alive at a
  transition). Filler work (remaining projections, W_o load, first half
  of the output projection) rides on non-transition units. Head order
  within a query block is [1, 0, 2, 3] so the final normalize is an odd
  head (its den needs no DMA hop off p64).
"""

import numpy as np

import concourse.bass as bass
import concourse.mybir as mybir
import concourse.tile as tile
from concourse import bacc
from concourse.bass_utils import run_bass_kernel_spmd

B, T, D, OUT, H = 2, 2048, 1024, 1024, 16
DO = 256            # output columns per core (4 heads x 64)
DEPTH = 64
NH = 4              # heads per core
KT = D // 128       # 8 contraction tiles for the projections
TT = T // 128       # 16 key tiles
NB = T // 512       # 4 query/time blocks
F32 = mybir.dt.float32
F16 = mybir.dt.float16
BF16 = mybir.dt.bfloat16
MMDT = BF16
EXP = mybir.ActivationFunctionType.Exp
MULT = mybir.AluOpType.mult
ADD = mybir.AluOpType.add

H_ORDER = [1, 0, 2, 3]   # emission order of heads within a query block

_CACHE = {}


def build_attention(nc, dbg=None):
    # x is sent chunk-contiguous: [nb, d, c] so each [128, 512] chunk DMA
    # is one contiguous 128KB transfer
    xt = nc.declare_dram_parameter("xt", [NB * D, 512], MMDT, isOutput=False)
    # weights are pre-arranged on the host into the SBUF layout so the
    # DMAs are contiguous full-rate transfers
    wq = nc.declare_dram_parameter("wq", [128, KT * DO], MMDT, isOutput=False)
    wk = nc.declare_dram_parameter("wk", [128, KT * DO], MMDT, isOutput=False)
    wv = nc.declare_dram_parameter("wv", [128, KT * DO], MMDT, isOutput=False)
    wo = nc.declare_dram_parameter("wo", [128, 2 * OUT], MMDT, isOutput=False)
    bq2 = nc.declare_dram_parameter("bq2", [128, 2], F32, isOutput=False)
    bv = nc.declare_dram_parameter("bv", [DO], F32, isOutput=False)
    bcol = nc.declare_dram_parameter("bcol", [128, TT], F32, isOutput=False)
    # output partials are chunk-contiguous too: [tb, nt-rows, 512]
    outT = nc.declare_dram_parameter("outT", [NB * OUT, 512], F16, isOutput=True)

    with tile.TileContext(nc) as tc:
        with (
            tc.tile_pool(name="cw", bufs=1) as cw,
            tc.tile_pool(name="stage", bufs=6) as stage,
            tc.tile_pool(name="persist", bufs=1) as persist,
            tc.tile_pool(name="small", bufs=2) as small,
            tc.tile_pool(name="ptp", bufs=6) as ptp,
            tc.tile_pool(name="px", bufs=1) as px,
            tc.tile_pool(name="ps_s", bufs=2, space="PSUM") as ps_s,
            tc.tile_pool(name="ps_mm", bufs=2, space="PSUM") as ps_mm,
        ):
            # ---- warmup: wake the PE HAM clock gate and the ACT exp table
            # while the input DMAs are in flight ----
            ones_f = cw.tile([128, NH], F32, tag="ones")
            nc.vector.memset(ones_f[:], 1.0)
            warm_w = cw.tile([128, 256], MMDT, tag="warmw")
            nc.vector.memset(warm_w[:], 0.0)
            warm_ps = ps_s.tile([128, 256], F32, tag="s", name="warm_ps")
            for _ in range(28):
                nc.tensor.matmul(warm_ps[:, :256], warm_w[:, :128], warm_w[:, :256],
                                 start=True, stop=True)
            warm_pt = cw.tile([128, NH], MMDT, tag="warmpt")
            nc.scalar.activation(warm_pt[:], ones_f[:], EXP, scale=1.0)

            # ---- inputs; x tiles split in column halves across two DMA
            # queues so the first projections start ~3us in ----
            def load_bf16(pool, dram_ap, shape, tag, eng=None):
                r = pool.tile(shape, MMDT, tag=tag, name=f"r_{tag}")
                (eng or nc.sync).dma_start(out=r[:], in_=dram_ap)
                return r

            wk_r = load_bf16(px, wk[:, :].rearrange("p (kt m) -> p kt m", m=DO), [128, KT, DO], "wk")
            wq_r = load_bf16(px, wq[:, :].rearrange("p (kt m) -> p kt m", m=DO), [128, KT, DO], "wq", eng=nc.scalar)
            # x tiles arrive in 512-column chunks, nb-major, split across
            # two DMA queues, so the first projection chains finish while
            # the bulk of x is still in flight
            xr = [px.tile([128, T], MMDT, tag=f"xr{kt}", name=f"r_xr{kt}") for kt in range(KT)]
            for nb in range(NB):
                csl = slice(nb * 512, (nb + 1) * 512)
                for kt in range(KT):
                    eng = nc.sync if kt % 2 == 0 else nc.scalar
                    eng.dma_start(out=xr[kt][:, csl],
                                  in_=xt[nb * D + kt * 128:nb * D + (kt + 1) * 128, :])

            # ---- gpsimd queue: biases first (they gate the projection
            # copies), kth zero-halves (gate the first S stationary), wv,
            # then PV-tile zeroing and wo ----
            qt2 = [persist.tile([128, T], MMDT, tag=f"qt{mi}", name=f"qt{mi}") for mi in range(2)]
            kth = [persist.tile([128, T], MMDT, tag=f"kh{h}", name=f"kh{h}") for h in range(NH)]
            vp = persist.tile([128, TT, NH * 128], MMDT, tag="vp")
            at2p = [persist.tile([128, T], MMDT, tag=f"atp{p}", name=f"atp{p}") for p in range(2)]
            bq_sb = cw.tile([128, 2], F32, tag="bq")
            nc.gpsimd.dma_start(out=bq_sb[:], in_=bq2[:, :])
            for h in H_ORDER:
                lo, hi = ((64, 128) if h % 2 == 0 else (0, 64))
                nc.gpsimd.memset(kth[h][lo:hi, :], 0.0)
            wv_r = load_bf16(px, wv[:, :].rearrange("p (kt m) -> p kt m", m=DO), [128, KT, DO], "wv", eng=nc.gpsimd)
            bcol_sb = cw.tile([128, TT], F32, tag="bcol")
            nc.gpsimd.dma_start(out=bcol_sb[:], in_=bcol[:, :])
            bv_sb = cw.tile([128, DO], F32, tag="bv")
            bv_ap = bv.ap()
            bv_bcast = bass.AP(tensor=bv_ap.tensor, offset=bv_ap.offset, ap=[[0, 128], [1, DO]])
            nc.gpsimd.dma_start(out=bv_sb[:], in_=bv_bcast)
            # PV stationary: [kt, head, 128 cols]; per-head column placement
            # is parity-asymmetric (see module docstring)
            nc.gpsimd.memset(vp[:, 0:4, :], 0.0)
            nc.gpsimd.memset(vp[:, 4:TT, :], 0.0)
            wo_r = load_bf16(px, wo[:, :].rearrange("p (j n) -> p j n", j=2), [128, 2, OUT], "wo", eng=nc.gpsimd)

            # ---- emission helpers (advanced by the interleaver) ----
            def q_group(mi, nb, pool_tag="attn"):
                """One [128,512] Q projection psum group: 8 matmuls + biased copy."""
                pool = ps_s if pool_tag == "s" else ps_mm
                ps = pool.tile([128, 1024], F32, tag=pool_tag, name=f"ps_q{mi}_{nb}")
                for kt in range(KT):
                    nc.tensor.matmul(
                        ps[:, :512],
                        wq_r[:, kt, mi * 128:(mi + 1) * 128],
                        xr[kt][:, nb * 512:(nb + 1) * 512],
                        start=(kt == 0),
                        stop=(kt == KT - 1),
                    )
                nc.vector.tensor_scalar_add(
                    qt2[mi][:, nb * 512:(nb + 1) * 512], ps[:, :512], bq_sb[:, mi:mi + 1]
                )

            def k_group(mi, nb, pool_tag="attn"):
                """K projection group, split-written into the two kth tiles."""
                pool = ps_s if pool_tag == "s" else ps_mm
                ps = pool.tile([128, 1024], F32, tag=pool_tag, name=f"ps_k{mi}_{nb}")
                for kt in range(KT):
                    nc.tensor.matmul(
                        ps[:, :512],
                        wk_r[:, kt, mi * 128:(mi + 1) * 128],
                        xr[kt][:, nb * 512:(nb + 1) * 512],
                        start=(kt == 0),
                        stop=(kt == KT - 1),
                    )
                sl = slice(nb * 512, (nb + 1) * 512)
                nc.vector.tensor_scalar_add(
                    kth[2 * mi][0:64, sl], ps[0:64, :512], bq_sb[0:64, mi:mi + 1]
                )
                nc.vector.tensor_scalar_add(
                    kth[2 * mi + 1][64:128, sl], ps[64:128, :512], bq_sb[64:128, mi:mi + 1]
                )

            def v_group(tt):
                ps = ps_mm.tile([128, 1024], F32, tag="attn", name=f"ps_v{tt}")
                for kt in range(KT):
                    nc.tensor.matmul(
                        ps[:, :DO],
                        xr[kt][:, tt * 128:(tt + 1) * 128],
                        wv_r[:, kt, :],
                        start=(kt == 0),
                        stop=(kt == KT - 1),
                    )
                vpt = vp[:, tt, :].rearrange("p (h c) -> p h c", c=128)
                # even heads (0,2): V at cols 0:64; odd heads (1,3): cols 64:128
                nc.vector.tensor_tensor(
                    vpt[:, 0::2, 0:64],
                    ps[:, :DO].rearrange("p (h c) -> p h c", c=64)[:, 0::2, :],
                    bv_sb[:, :].rearrange("p (h c) -> p h c", c=64)[:, 0::2, :],
                    ADD,
                )
                nc.vector.tensor_tensor(
                    vpt[:, 1::2, 64:128],
                    ps[:, :DO].rearrange("p (h c) -> p h c", c=64)[:, 1::2, :],
                    bv_sb[:, :].rearrange("p (h c) -> p h c", c=64)[:, 1::2, :],
                    ADD,
                )
                nc.gpsimd.tensor_copy(out=vpt[:, 0::2, 64:65], in_=ones_f[:, 0:2, None])
                nc.gpsimd.tensor_copy(out=vpt[:, 1::2, 0:1], in_=ones_f[:, 2:4, None])

            def emit_pv(h, attn_ps, kt, pt):
                for half in range(2):
                    nc.tensor.matmul(
                        attn_ps[:, half * 512:(half + 1) * 512],
                        vp[:, kt, h * 128:(h + 1) * 128],
                        pt[:, half * 512:(half + 1) * 512],
                        start=(kt == 0),
                        stop=(kt == TT - 1),
                    )

            def normalize(qbp, h, attn_ps, split=False):
                sl = slice(qbp * 1024, (qbp + 1) * 1024)
                if h % 2 == 0:
                    # den at psum p64; move to p0 for recip+broadcast
                    den = cw.tile([65, 1024], F32, tag="den", name=f"den{qbp}_{h}")
                    nc.vector.tensor_copy(out=den[64:65, :], in_=attn_ps[64:65, :])
                    d0 = cw.tile([1, 1024], F32, tag="d0", name=f"d0{qbp}_{h}")
                    nc.sync.dma_start(out=d0[:], in_=den[64:65, :])
                    rec = cw.tile([1, 1024], F32, tag="rec", name=f"rec{qbp}_{h}")
                    nc.vector.reciprocal_approx_fast(rec[:], d0[:])
                    rb = small.tile([64, 1024], F32, tag="rb", name=f"rb{qbp}_{h}")
                    nc.gpsimd.partition_broadcast(rb[:], rec[:])
                    nc.vector.tensor_tensor(
                        at2p[h // 2][0:64, sl], attn_ps[0:64, :], rb[:], MULT
                    )
                else:
                    # den already at psum p0: recip straight off psum.
                    # split=True pipelines the two 512-column halves so the
                    # tail output projection can start ~2.5us sooner.
                    rbh = small.tile([128, 1024], F32, tag="rbh", name=f"rbh{qbp}_{h}")
                    halves = ((0, 1024),) if not split else ((0, 512), (512, 1024))
                    for lo, hi in halves:
                        rec = cw.tile([1, 1024], F32, tag="rec", name=f"rec{qbp}_{h}_{lo}")
                        nc.vector.reciprocal_approx_fast(rec[:, 0:hi - lo], attn_ps[0:1, lo:hi])
                        nc.gpsimd.partition_broadcast(rbh[:, lo:hi], rec[:, 0:hi - lo])
                        nc.vector.tensor_tensor(
                            at2p[h // 2][64:128, qbp * 1024 + lo:qbp * 1024 + hi],
                            attn_ps[64:128, lo:hi], rbh[64:128, lo:hi], MULT
                        )

            def c_group(nt, tb, evac=None, pool=None):
                ps = (pool or ps_mm).tile([128, 1024], F32, tag=("s" if pool is ps_s else "attn"), name=f"ps_c{nt}_{tb}")
                for j in range(2):
                    nc.tensor.matmul(
                        ps[:, :512],
                        wo_r[:, j, nt * 128:(nt + 1) * 128],
                        at2p[j][:, tb * 512:(tb + 1) * 512],
                        start=(j == 0),
                        stop=(j == 1),
                    )
                o_sb = stage.tile([128, 512], F16, tag="stage", name="o_sb")
                if evac == "s":
                    nc.scalar.copy(o_sb[:], ps[:, :512])
                else:
                    nc.vector.tensor_copy(out=o_sb[:], in_=ps[:, :512])
                nc.sync.dma_start(
                    out=outT[tb * OUT + nt * 128:tb * OUT + (nt + 1) * 128, :],
                    in_=o_sb[:],
                )

            # ---- emission schedule ----
            # minimal upfront work for the first unit, then ONE globally
            # software-pipelined stream over all 128 attention units.
            # prep: everything the first S unit needs, plus V groups whose
            # x columns arrive early (they run inside the x DMA wait)
            k_group(0, 0, pool_tag="s")
            q_group(0, 0, pool_tag="s")
            q_group(0, 1, pool_tag="s")
            for tt in range(4):
                v_group(tt)

            # v_group(tt) must be emitted >=1 unit before its PV consumer
            # (PV for kt=tt is emitted at idx tt+1); K(0,nb) before idx 4nb.
            era1 = [
                [lambda: v_group(4)], [lambda: v_group(5)],
                [lambda: k_group(0, 1)],
                [lambda: v_group(6)], [lambda: v_group(7)], [lambda: v_group(8)],
                [lambda: k_group(0, 2)],
                [lambda: v_group(9)], [lambda: v_group(10)], [lambda: v_group(11)],
                [lambda: k_group(0, 3)],
                [lambda: v_group(12)], [lambda: v_group(13)],
                [lambda: v_group(14)], [lambda: v_group(15)],
                None,
            ]
            era2 = [None] * 16
            era2[1] = [lambda: k_group(1, 0)]
            era2[3] = [lambda: k_group(1, 1)]
            era2[5] = [lambda: k_group(1, 2)]
            era2[7] = [lambda: k_group(1, 3)]
            era2[9] = [lambda: q_group(1, 0)]
            era2[11] = [lambda: q_group(1, 1)]
            era3 = [None] * 32
            era3[2] = [lambda: q_group(0, 2)]
            era3[6] = [lambda: q_group(0, 3)]
            era3[10] = [lambda: q_group(1, 2)]
            era3[14] = [lambda: q_group(1, 3)]
            c_work = [(nt, tb) for tb in range(2) for nt in range(OUT // 128)]

            units = [(qbp, h, kt) for qbp in range(2) for h in H_ORDER for kt in range(TT)]
            attn_tiles = {}
            prev = None
            for idx, (qbp, h, kt) in enumerate(units):
                if kt == 0:
                    attn_tiles[(qbp, h)] = ps_mm.tile(
                        [128, 1024], F32, tag="attn", name=f"attn_{qbp}_{h}"
                    )
                s_ps = ps_s.tile([128, 1024], F32, tag="s", name=f"s_{qbp}_{h}_{kt}")
                for half in range(2):
                    nc.tensor.matmul(
                        s_ps[:, half * 512:(half + 1) * 512],
                        kth[h][:, kt * 128:(kt + 1) * 128],
                        qt2[h // 2][:, qbp * 1024 + half * 512:qbp * 1024 + (half + 1) * 512],
                        start=True,
                        stop=True,
                    )
                pt = ptp.tile([128, 1024], MMDT, tag="pt")
                nc.scalar.activation(
                    pt[:], s_ps[:], EXP, bias=bcol_sb[:, kt:kt + 1], scale=0.125
                )
                if prev is not None:
                    pq, ph, pk, ppt = prev
                    emit_pv(ph, attn_tiles[(pq, ph)], pk, ppt)
                    if pk == TT - 1:
                        normalize(pq, ph, attn_tiles.pop((pq, ph)))
                # filler work, away from head-transition units
                if idx < 16:
                    for item in era1[idx] or []:
                        item()
                elif idx < 32:
                    for item in era2[idx - 16] or []:
                        item()
                elif idx < 64:
                    for item in era3[idx - 32] or []:
                        item()
                elif idx >= 64 and 1 <= kt <= 14 and kt % 3 == 1 and c_work:
                    nt, tb = c_work.pop(0)
                    c_group(nt, tb)
                prev = (qbp, h, kt, pt)
            pq, ph, pk, ppt = prev
            emit_pv(ph, attn_tiles[(pq, ph)], pk, ppt)
            normalize(pq, ph, attn_tiles.pop((pq, ph)), split=True)

            while c_work:
                nt, tb = c_work.pop(0)
                c_group(nt, tb)
            # tail: ScalarE is idle now -- alternate psum evacuation between
            # the vector and scalar engines so the matmul stream never waits
            for i, (nt, tb) in enumerate([(nt, tb) for tb in range(2, NB) for nt in range(OUT // 128)]):
                c_group(nt, tb, evac=("s" if i % 2 == 0 else "v"),
                        pool=(ps_s if i % 2 == 0 else ps_mm))

            if dbg:
                for mi in range(2):
                    nc.sync.dma_start(out=dbg["d_qt"][mi][:, :], in_=qt2[mi][:])
                for h in range(NH):
                    nc.sync.dma_start(out=dbg["d_kt"][h][:, :], in_=kth[h][:])
                for j in range(2):
                    nc.sync.dma_start(out=dbg["d_at"][j][:, :], in_=at2p[j][:])
                nc.sync.dma_start(out=dbg["d_vp"][:, :, :], in_=vp[:])


def _build():
    nc = bacc.Bacc(trn_type="TRN2")
    build_attention(nc)
    nc.compile()
    return nc


def _get_nc():
    if "nc" not in _CACHE:
        _CACHE["nc"] = _build()
    return _CACHE["nc"]


def make_in_maps(x, W_q, b_q, W_k, W_v, b_v, W_o, bias):
    import ml_dtypes
    bf16 = ml_dtypes.bfloat16

    def warr(w):
        # [D, DO] -> SBUF layout [128, KT*DO] (partition-major, kt-tiled)
        return np.ascontiguousarray(
            w.reshape(KT, 128, DO).transpose(1, 0, 2).reshape(128, KT * DO))

    def woarr(w):
        # [2*128, OUT] -> [two*64+p, j, n] -> [128, 2*OUT]
        return np.ascontiguousarray(
            w.reshape(2, 2, 64, OUT).transpose(1, 2, 0, 3).reshape(128, 2 * OUT))

    in_maps = []
    xtb = [np.ascontiguousarray(
        x[b].T.astype(bf16).reshape(D, NB, 512).transpose(1, 0, 2).reshape(NB * D, 512))
        for b in range(B)]
    wqb = W_q.astype(bf16)
    wkb = W_k.astype(bf16)
    wvb = W_v.astype(bf16)
    wob = W_o.astype(bf16)
    for c in range(8):
        b, hg = divmod(c, 4)
        sl = slice(hg * DO, (hg + 1) * DO)
        in_maps.append({
            "xt": xtb[b],
            "wq": warr(wqb[:, sl]),
            "wk": warr(wkb[:, sl]),
            "wv": warr(wvb[:, sl]),
            "wo": woarr(wob[sl, :]),
            "bq2": np.ascontiguousarray(b_q[sl].reshape(2, 128).T),
            "bv": np.ascontiguousarray(b_v[sl]),
            "bcol": np.ascontiguousarray(bias.reshape(TT, 128).T),
        })
    return in_maps


def kernel(x, W_q, b_q, W_k, b_k, W_v, b_v, W_o, b_o, bias, **_ignored):
    x = np.asarray(x, dtype=np.float32)
    W_q = np.asarray(W_q, dtype=np.float32)
    W_k = np.asarray(W_k, dtype=np.float32)
    W_v = np.asarray(W_v, dtype=np.float32)
    W_o = np.asarray(W_o, dtype=np.float32)
    b_q = np.asarray(b_q, dtype=np.float32)
    b_v = np.asarray(b_v, dtype=np.float32)
    b_o = np.asarray(b_o, dtype=np.float32)
    bias = np.asarray(bias, dtype=np.float32)

    nc = _get_nc()
    in_maps = make_in_maps(x, W_q, b_q, W_k, W_v, b_v, W_o, bias)
    _CACHE["in_maps"] = in_maps
    res = run_bass_kernel_spmd(nc, in_maps, list(range(8)))
    out = np.zeros((B, T, OUT), dtype=np.float32)
    for c in range(8):
        oc = res.results[c]["outT"].reshape(NB, OUT, 512)
        out[c // 4] += np.concatenate([oc[tb] for tb in range(NB)], axis=1).T.astype(np.float32)
    out += b_o
    return out
